# revision 1
# baseline (speedup 1.0000x reference)
"""v2: big-FD batched kernel — one instruction per stage covering all 16 heads.

Layout per chunk (k-major big tiles [P, KC, H], col = k*16+h):
  h-major big tiles [P, H, KC] (contiguous per-h rows):
  DVE premask-extract: id_im = raw[:,:,2i] * m32  (4 small ops)
  ACT: m32 cast; m1b[:,h,:] = id1m * p_h (16 contiguous ops, exact fp32)
  GP:  m2b[:,h,:] = id2m * p2_h-bcast, m3b likewise (32 tt ops, exact int32)
  DVE fold chain (stt fuses mod+fold; masking distributes over xor so late
  masks kill earlier garbage):
       y1 = (m3b & M) ^ m2b          (one op, FD=16*KC)
       y2 = (y1 & M) ^ m1b           (one op)
       out[h,k] = (y2 & M) ^ id0m    (16 per-h ops, strided write)
"""
import sys

for _p in ("/opt/trn_rl_repo", "/root/.axon_site/_ro/trn_rl_repo"):
    if _p not in sys.path:
        sys.path.append(_p)

import numpy as np

B, S, O, H = 64, 8192, 4, 16
NCORES = 8
BPC = B // NCORES
N = BPC * S
P = 128
KTOT = N // P                  # 512
KC = 256
NCH = KTOT // KC               # 2
TABLE = 1 << 20
MASK20 = TABLE - 1

_cache = {}


def _build(p1, p2, p3, iters=1):
    import concourse.bass as bass
    from concourse import mybir

    A = mybir.AluOpType
    I32 = mybir.dt.int32
    U8 = mybir.dt.uint8

    nc = bass.Bass()

    ids_d = nc.declare_dram_parameter("ids", [P, KTOT, 8], I32, isOutput=False)
    msk_d = nc.declare_dram_parameter("msk", [P, KTOT], U8, isOutput=False)
    cst_d = nc.declare_dram_parameter("cst", [P, 2 * H], I32, isOutput=False)
    out_d = nc.declare_dram_parameter("out", [P, KTOT, 2 * H], I32, isOutput=True)

    raw = [nc.alloc_sbuf_tensor(f"raw{c}", [P, KC, 8], I32) for c in range(NCH)]
    mk8 = [nc.alloc_sbuf_tensor(f"mk8{c}", [P, KC], U8) for c in range(NCH)]
    m32 = [nc.alloc_sbuf_tensor(f"m32{c}", [P, KC], I32) for c in range(NCH)]
    idm = [[nc.alloc_sbuf_tensor(f"id{i}m{c}", [P, KC], I32) for i in range(4)] for c in range(NCH)]
    cst = nc.alloc_sbuf_tensor("cst_t", [P, 2 * H], I32)
    mA = nc.alloc_sbuf_tensor("mA", [P, 1], I32)          # 0xFFFFF per partition
    m1b = [nc.alloc_sbuf_tensor(f"m1b{c}", [P, H, KC], I32) for c in range(NCH)]
    m2b = [nc.alloc_sbuf_tensor(f"m2b{c}", [P, H, KC], I32) for c in range(NCH)]
    m3b = [nc.alloc_sbuf_tensor(f"m3b{c}", [P, H, KC], I32) for c in range(NCH)]
    f1b = nc.alloc_sbuf_tensor("f1b", [P, H, KC], I32)
    ot = [nc.alloc_sbuf_tensor(f"ot{c}", [P, KC, 2 * H], I32) for c in range(NCH)]

    s_in = nc.alloc_semaphore("s_in")
    s_msk = nc.alloc_semaphore("s_msk")
    s_idm = nc.alloc_semaphore("s_idm")
    s_m1 = nc.alloc_semaphore("s_m1")
    s_m2 = nc.alloc_semaphore("s_m2")
    s_m3 = nc.alloc_semaphore("s_m3")
    s_f = nc.alloc_semaphore("s_f")
    s_out = nc.alloc_semaphore("s_out")



    with nc.Block() as block:
        @block.sync
        def _(sync: bass.BassEngine):
            sync.dma_start(out=cst[:], in_=cst_d[:]).then_inc(s_in, 16)
            for r in range(iters):
                if r > 0:
                    sync.wait_ge(s_out, 96 * NCH * r)
                for c in range(NCH):
                    sync.dma_start(out=raw[c][:], in_=ids_d[:, c * KC:(c + 1) * KC, :]).then_inc(s_in, 16)
                    sync.dma_start(out=mk8[c][:], in_=msk_d[:, c * KC:(c + 1) * KC]).then_inc(s_in, 16)
                for c in range(NCH):
                    nq = 8 if c == NCH - 1 else 4
                    off = 12 * r + (0 if c == 0 else 4)
                    for q in range(nq):
                        kq = KC // nq
                        sync.wait_ge(s_f, off + q + 1)
                        sync.dma_start(out=out_d[:, c * KC + q * kq:c * KC + (q + 1) * kq, :],
                                       in_=ot[c][:, q * kq:(q + 1) * kq, :]).then_inc(s_out, 16)
            sync.wait_ge(s_out, 96 * NCH * iters)

        @block.scalar
        def _(sc: bass.BassEngine):
            for c in range(NCH):
                sc.memzero(ot[c][:])
            for r in range(iters):
                for c in range(NCH):
                    t = NCH * r + c
                    sc.wait_ge(s_idm, 2 * t + 2)
                    for h in range(H):
                        ins = sc.mul(m1b[c][:, h, :], idm[c][1][:], float(p1[h]))
                        if h == H - 1:
                            ins.then_inc(s_m1, 1)

        @block.gpsimd
        def _(gp: bass.BassEngine):
            for r in range(iters):
                for c in range(NCH):
                    t = NCH * r + c
                    gp.wait_ge(s_idm, 2 * t + 1)
                    for h in range(H):
                        ins = gp.tensor_tensor(m2b[c][:, h, :], idm[c][2][:],
                                               cst[:, h:h + 1].broadcast_to([P, KC]), A.mult)
                        if h == H - 1:
                            ins.then_inc(s_m2, 1)
                    for h in range(H):
                        ins = gp.tensor_tensor(m3b[c][:, h, :], idm[c][3][:],
                                               cst[:, H + h:H + h + 1].broadcast_to([P, KC]), A.mult)
                        if h == H - 1:
                            ins.then_inc(s_m3, 1)

        @block.vector
        def _(v: bass.BassEngine):
            v.memset(mA[:], MASK20)
            for r in range(iters):
                # front-load both chunks' premasks so GP/ACT start early
                for c in range(NCH):
                    t = NCH * r + c
                    v.wait_ge(s_in, 16 + 32 * NCH * r + 32 * (c + 1))
                    v.tensor_tensor(idm[c][2][:], raw[c][:, :, 4], mk8[c][:], A.mult)
                    v.tensor_tensor(idm[c][3][:], raw[c][:, :, 6], mk8[c][:], A.mult).then_inc(s_idm, 1)
                    v.tensor_tensor(idm[c][1][:], raw[c][:, :, 2], mk8[c][:], A.mult).then_inc(s_idm, 1)
                    v.tensor_tensor(idm[c][0][:], raw[c][:, :, 0], mk8[c][:], A.mult)
                for c in range(NCH):
                    t = NCH * r + c
                    v.wait_ge(s_m2, t + 1)
                    v.wait_ge(s_m3, t + 1)
                    v.scalar_tensor_tensor(f1b[:], m3b[c][:], mA[:], m2b[c][:],
                                           A.bitwise_and, A.bitwise_xor)
                    v.wait_ge(s_m1, t + 1)
                    v.scalar_tensor_tensor(m3b[c][:], f1b[:], mA[:], m1b[c][:],
                                           A.bitwise_and, A.bitwise_xor)
                    nq = 8 if c == NCH - 1 else 4
                    kq = KC // nq
                    for q in range(nq):
                        id0q = idm[c][0][:, q * kq:(q + 1) * kq].rearrange(
                            "p (x k) -> p x k", x=1).broadcast_to([P, H, kq])
                        out_ap = ot[c][:, q * kq:(q + 1) * kq, 0:2 * H:2].rearrange("p k h -> p h k")
                        v.scalar_tensor_tensor(out_ap, m3b[c][:, :, q * kq:(q + 1) * kq], mA[:],
                                               id0q, A.bitwise_and, A.bitwise_xor).then_inc(s_f, 1)

    return nc


def kernel(ngram_ids, ngram_mask, prime_powers, table_size):
    from concourse.bass_utils import run_bass_kernel_spmd

    ids = np.asarray(ngram_ids)
    msk = np.asarray(ngram_mask)
    pw = np.asarray(prime_powers)
    assert int(table_size) == TABLE
    assert ids.shape == (B, S, O) and ids.dtype == np.int64
    assert pw.shape[1] >= 4 and np.all(pw[:, 0] == 1)

    p1 = [int(x) for x in pw[:H, 1]]
    p2 = [int(x) for x in pw[:H, 2]]
    p3 = [int(x & 0xFFFFFFFF) for x in pw[:H, 3]]

    key = (tuple(p1), tuple(p2), tuple(p3))
    if key not in _cache:
        _cache[key] = _build(p1, p2, p3)
    nc = _cache[key]

    ids32 = ids.view(np.int32).reshape(B, S, 2 * O)
    msk8 = np.ascontiguousarray(msk).astype(np.uint8, copy=False)

    cstv = np.empty((P, 2 * H), np.int32)
    cstv[:, :H] = np.asarray(p2, np.int64).astype(np.int32)[None, :]
    cstv[:, H:] = np.asarray(p3, np.uint32).view(np.int32)[None, :]

    in_maps = []
    for c in range(NCORES):
        core_ids = np.ascontiguousarray(ids32[c * BPC:(c + 1) * BPC]).reshape(P, KTOT, 8)
        core_msk = np.ascontiguousarray(msk8[c * BPC:(c + 1) * BPC]).reshape(P, KTOT)
        in_maps.append({"ids": core_ids, "msk": core_msk, "cst": cstv})

    res = run_bass_kernel_spmd(nc, in_maps, list(range(NCORES)))

    out = np.empty((B, S, H), np.int64)
    for c in range(NCORES):
        o32 = res.results[c]["out"]
        out[c * BPC:(c + 1) * BPC] = o32.reshape(BPC, S, 2 * H).view(np.int64)
    return out


if __name__ == "__main__":
    rng = np.random.default_rng(0)
    ids = rng.integers(0, 32000, size=(B, S, O)).astype(np.int64)
    msk = np.ones((B, S), dtype=bool)
    msk[3, 100:200] = False  # exercise the mask path
    primes = np.array([31, 37, 41, 43, 47, 53, 59, 61, 67, 71, 73, 79, 83, 89, 97, 101], np.int64)
    pw = primes[:, None] ** np.arange(8, dtype=np.int64)[None, :]
    got = kernel(ids, msk, pw, TABLE)
    w = ids[:, :, :, None].astype(np.int64) * pw.T[:4][None, None, :, :]
    exp = w[..., 0, :]
    for i in range(1, 4):
        exp = exp ^ w[..., i, :]
    exp = (exp % TABLE) * msk[..., None]
    print("match:", np.array_equal(got, exp))
    bad = got != exp
    if bad.any():
        idx = np.argwhere(bad)
        print("nbad:", len(idx))
        for b_, s_, h_ in idx[:5]:
            print(b_, s_, h_, got[b_, s_, h_], exp[b_, s_, h_])



# revision 2
# speedup vs baseline: 1.2439x; 1.2439x over previous
"""v3: halved DMA traffic + big-op engine assignment.

Traffic: ids as low-32-bit words only [P,KTOT,4] i32 (1 MiB/core), mask u8
(64 KiB), output int32-only [P,KTOT,H] (4 MiB) — host widens to int64.

Per chunk (KC=256 cols, NCH=2):
  DVE premask: idm_i = raw[:,:,i] * mk8   (4 small tt ops; order 2,3,1,0)
  Pool: m2b = idm2-bcast * p2-bcast, m3b likewise (2 big [P,H,KC] tt ops,
        integer-exact path)
  ACT:  m1b[:,h,:] = idm1 * p1[h]         (16 ops, fp32-exact: products < 2^24)
  DVE fold chain (stt fuses and+xor; premask distributes over xor):
     f1 = (m3b & M) ^ m2b
     y  = (f1 & M) ^ m1b                  (written back into m3b)
     ot[:,k,h] = (y & M) ^ idm0-bcast     (4 q-quarter ops, transposed write)
"""
import sys

for _p in ("/opt/trn_rl_repo", "/root/.axon_site/_ro/trn_rl_repo"):
    if _p not in sys.path:
        sys.path.append(_p)

import numpy as np

B, S, O, H = 64, 8192, 4, 16
NCORES = 8
BPC = B // NCORES
N = BPC * S
P = 128
KTOT = N // P                  # 512
KC = 256
NCH = KTOT // KC               # 2
NQ = 4                         # output stores per chunk
KQ = KC // NQ
TABLE = 1 << 20
MASK20 = TABLE - 1

_cache = {}


def _build(p1, p2, p3, iters=1):
    import concourse.bass as bass
    from concourse import mybir

    A = mybir.AluOpType
    I32 = mybir.dt.int32
    U8 = mybir.dt.uint8

    nc = bass.Bass()

    ids_d = nc.declare_dram_parameter("ids", [P, KTOT, 4], I32, isOutput=False)
    msk_d = nc.declare_dram_parameter("msk", [P, KTOT], U8, isOutput=False)
    cst_d = nc.declare_dram_parameter("cst", [P, 3 * H], I32, isOutput=False)
    out_d = nc.declare_dram_parameter("out", [P, KTOT, H], I32, isOutput=True)

    raw = [nc.alloc_sbuf_tensor(f"raw{c}", [P, KC, 4], I32) for c in range(NCH)]
    mk8 = nc.alloc_sbuf_tensor("mk8", [P, KTOT], U8)
    idm = [[nc.alloc_sbuf_tensor(f"id{i}m{c}", [P, KC], I32) for i in range(4)] for c in range(NCH)]
    cst = nc.alloc_sbuf_tensor("cst_t", [P, 3 * H], I32)
    mA = nc.alloc_sbuf_tensor("mA", [P, 1], I32)
    m1b = [nc.alloc_sbuf_tensor(f"m1b{c}", [P, H, KC], I32) for c in range(NCH)]
    m2b = [nc.alloc_sbuf_tensor(f"m2b{c}", [P, H, KC], I32) for c in range(NCH)]
    m3b = [nc.alloc_sbuf_tensor(f"m3b{c}", [P, H, KC], I32) for c in range(NCH)]
    f1b = nc.alloc_sbuf_tensor("f1b", [P, H, KC], I32)
    ot = [nc.alloc_sbuf_tensor(f"ot{c}", [P, KC, H], I32) for c in range(NCH)]

    s_in = nc.alloc_semaphore("s_in")
    s_pm = nc.alloc_semaphore("s_pm")      # premask chunk done (raw/mk free)
    s_idm23 = nc.alloc_semaphore("s_idm23")
    s_idm1 = nc.alloc_semaphore("s_idm1")
    s_m1 = nc.alloc_semaphore("s_m1")
    s_m2 = nc.alloc_semaphore("s_m2")
    s_m3 = nc.alloc_semaphore("s_m3")
    s_f1 = nc.alloc_semaphore("s_f1")
    s_f2 = nc.alloc_semaphore("s_f2")
    s_f = nc.alloc_semaphore("s_f")        # +1 per q-quarter written
    s_out = nc.alloc_semaphore("s_out")    # +16 per store

    def t(r, c):
        return NCH * r + c

    with nc.Block() as block:
        @block.sync
        def _(sync: bass.BassEngine):
            sync.dma_start(out=cst[:], in_=cst_d[:]).then_inc(s_in, 16)
            for r in range(iters):
                # mask reload waits until both premask chunks of r-1 done
                if r > 0:
                    sync.wait_ge(s_pm, NCH * r)
                sync.dma_start(out=mk8[:], in_=msk_d[:]).then_inc(s_in, 16)
                for c in range(NCH):
                    if r > 0:
                        sync.wait_ge(s_pm, NCH * (r - 1) + c + 1)
                    sync.dma_start(out=raw[c][:], in_=ids_d[:, c * KC:(c + 1) * KC, :]).then_inc(s_in, 16)
                for c in range(NCH):
                    for q in range(NQ):
                        sync.wait_ge(s_f, NQ * t(r, c) + q + 1)
                        sync.dma_start(
                            out=out_d[:, c * KC + q * KQ:c * KC + (q + 1) * KQ, :],
                            in_=ot[c][:, q * KQ:(q + 1) * KQ, :],
                        ).then_inc(s_out, 16)
            sync.wait_ge(s_out, 16 * NQ * NCH * iters)

        @block.vector
        def _(v: bass.BassEngine):
            v.memset(mA[:], MASK20)
            for r in range(iters):
                for c in range(NCH):
                    # premask: order 2,3 (Pool), 1 (ACT), 0 (kept for f3)
                    v.wait_ge(s_in, 16 + 48 * r + 16 + 16 * (c + 1))
                    mseg = mk8[:, c * KC:(c + 1) * KC]
                    if r > 0:
                        # idm bufs free once f3(r-1,c) read idm0; f3 increments s_f
                        v.wait_ge(s_f, NQ * (t(r - 1, c) + 1))
                    v.tensor_tensor(idm[c][2][:], raw[c][:, :, 2], mseg, A.mult).then_inc(s_idm23, 1)
                    v.tensor_tensor(idm[c][3][:], raw[c][:, :, 3], mseg, A.mult).then_inc(s_idm23, 1)
                    v.tensor_tensor(idm[c][1][:], raw[c][:, :, 1], mseg, A.mult).then_inc(s_idm1, 1)
                    v.tensor_tensor(idm[c][0][:], raw[c][:, :, 0], mseg, A.mult).then_inc(s_pm, 1)
                for c in range(NCH):
                    v.wait_ge(s_m2, t(r, c) + 1)
                    v.wait_ge(s_m3, t(r, c) + 1)
                    v.scalar_tensor_tensor(f1b[:], m3b[c][:], mA[:], m2b[c][:],
                                           A.bitwise_and, A.bitwise_xor).then_inc(s_f1, 1)
                    v.wait_ge(s_m1, t(r, c) + 1)
                    v.scalar_tensor_tensor(m3b[c][:], f1b[:], mA[:], m1b[c][:],
                                           A.bitwise_and, A.bitwise_xor).then_inc(s_f2, 1)
                    for q in range(NQ):
                        if r > 0:
                            v.wait_ge(s_out, 16 * (NQ * t(r - 1, c) + q + 1))
                        id0q = idm[c][0][:, q * KQ:(q + 1) * KQ].rearrange(
                            "p (x k) -> p x k", x=1).broadcast_to([P, H, KQ])
                        out_ap = ot[c][:, q * KQ:(q + 1) * KQ, :].rearrange("p k h -> p h k")
                        v.scalar_tensor_tensor(out_ap, m3b[c][:, :, q * KQ:(q + 1) * KQ], mA[:],
                                               id0q, A.bitwise_and, A.bitwise_xor).then_inc(s_f, 1)

        @block.scalar
        def _(sc: bass.BassEngine):
            for r in range(iters):
                for c in range(NCH):
                    sc.wait_ge(s_idm1, t(r, c) + 1)
                    if r > 0:
                        sc.wait_ge(s_f2, t(r - 1, c) + 1)
                    for h in range(H):
                        ins = sc.mul(m1b[c][:, h, :], idm[c][1][:], float(p1[h]))
                        if h == H - 1:
                            ins.then_inc(s_m1, 1)

        @block.gpsimd
        def _(gp: bass.BassEngine):
            for r in range(iters):
                for c in range(NCH):
                    gp.wait_ge(s_idm23, 2 * t(r, c) + 1)
                    if r > 0:
                        gp.wait_ge(s_f1, t(r - 1, c) + 1)
                    i2b = idm[c][2][:].rearrange("p (x k) -> p x k", x=1).broadcast_to([P, H, KC])
                    c2b = cst[:, 0:H].rearrange("p (h x) -> p h x", x=1).broadcast_to([P, H, KC])
                    gp.tensor_tensor(m2b[c][:], i2b, c2b, A.mult).then_inc(s_m2, 1)
                    gp.wait_ge(s_idm23, 2 * t(r, c) + 2)
                    if r > 0:
                        gp.wait_ge(s_f, NQ * (t(r - 1, c) + 1))
                    i3b = idm[c][3][:].rearrange("p (x k) -> p x k", x=1).broadcast_to([P, H, KC])
                    c3b = cst[:, H:2 * H].rearrange("p (h x) -> p h x", x=1).broadcast_to([P, H, KC])
                    gp.tensor_tensor(m3b[c][:], i3b, c3b, A.mult).then_inc(s_m3, 1)

    return nc


def _prep(ngram_ids, ngram_mask, prime_powers):
    """Shared host-side prep: per-core input maps + prime constants."""
    ids = np.asarray(ngram_ids)
    msk = np.asarray(ngram_mask)
    pw = np.asarray(prime_powers)

    p1 = [int(x) for x in pw[:H, 1]]
    p2 = [int(x) for x in pw[:H, 2]]
    p3 = [int(x & 0xFFFFFFFF) for x in pw[:H, 3]]

    ids32 = ids.view(np.int32).reshape(B, S, 2 * O)[:, :, 0::2]   # low words
    msk8 = np.ascontiguousarray(msk).astype(np.uint8, copy=False)

    cstv = np.empty((P, 3 * H), np.int32)
    cstv[:, :H] = np.asarray(p2, np.int64).astype(np.int32)[None, :]
    cstv[:, H:2 * H] = np.asarray(p3, np.uint32).view(np.int32)[None, :]
    cstv[:, 2 * H:] = np.asarray(p1, np.int64).astype(np.int32)[None, :]

    in_maps = []
    for c in range(NCORES):
        core_ids = np.ascontiguousarray(ids32[c * BPC:(c + 1) * BPC]).reshape(P, KTOT, 4)
        core_msk = np.ascontiguousarray(msk8[c * BPC:(c + 1) * BPC]).reshape(P, KTOT)
        in_maps.append({"ids": core_ids, "msk": core_msk, "cst": cstv})
    return in_maps, p1, p2, p3


def kernel(ngram_ids, ngram_mask, prime_powers, table_size):
    from concourse.bass_utils import run_bass_kernel_spmd

    assert int(table_size) == TABLE
    ids = np.asarray(ngram_ids)
    pw = np.asarray(prime_powers)
    assert ids.shape == (B, S, O) and ids.dtype == np.int64
    assert pw.shape[1] >= 4 and np.all(pw[:, 0] == 1)

    in_maps, p1, p2, p3 = _prep(ngram_ids, ngram_mask, prime_powers)

    key = (tuple(p1), tuple(p2), tuple(p3))
    if key not in _cache:
        _cache[key] = _build(p1, p2, p3)
    nc = _cache[key]

    res = run_bass_kernel_spmd(nc, in_maps, list(range(NCORES)))

    out = np.empty((B, S, H), np.int64)
    for c in range(NCORES):
        o32 = res.results[c]["out"]
        out[c * BPC:(c + 1) * BPC] = o32.reshape(BPC, S, H).astype(np.int64)
    return out


if __name__ == "__main__":
    rng = np.random.default_rng(0)
    ids = rng.integers(0, 32000, size=(B, S, O)).astype(np.int64)
    msk = np.ones((B, S), dtype=bool)
    msk[3, 100:200] = False
    primes = np.array([31, 37, 41, 43, 47, 53, 59, 61, 67, 71, 73, 79, 83, 89, 97, 101], np.int64)
    pw = primes[:, None] ** np.arange(8, dtype=np.int64)[None, :]
    got = kernel(ids, msk, pw, TABLE)
    w = ids[:, :, :, None].astype(np.int64) * pw.T[:4][None, None, :, :]
    exp = w[..., 0, :]
    for i in range(1, 4):
        exp = exp ^ w[..., i, :]
    exp = (exp % TABLE) * msk[..., None]
    print("match:", np.array_equal(got, exp))
    bad = got != exp
    if bad.any():
        idx = np.argwhere(bad)
        print("nbad:", len(idx))
        for b_, s_, h_ in idx[:5]:
            print(b_, s_, h_, got[b_, s_, h_], exp[b_, s_, h_])
